# revision 1
# baseline (speedup 1.0000x reference)
"""Trainium2 Bass kernel for a GINE message-passing layer.

Reference computation (N=100000 nodes, E=600000 edges, D=128):
    msg  = relu(x[src] + edge_attr)            # [E, D]
    aggr = segment_sum(msg, dst, N)            # [N, D]
    z    = (1 + eps) * x + aggr
    h    = relu(bn1(z @ W1.T + b1)) @ W2.T + b2
    out  = relu(bn2(x + h))

Distribution strategy (8 NeuronCores, host-side shard/unshard):
  * Nodes are partitioned across the 8 cores (graph parallel).  Node->core
    and node->window assignment is degree-balanced (LPT) so every 128-node
    "window" of each core receives a near-equal number of incoming edges.
  * Edges are assigned to the core that owns their destination node, so the
    segment-sum is core-local.  Every core keeps the full gather table
    (src-chunked copy of x) in its HBM and gathers x[src] rows with the
    SWDGE dma_gather instruction (the "halo" is read on demand - full input
    replication makes the all-gather a host-side copy).
  * dma_gather indices are int16, so the gather table is split into 4
    chunks of 25088 rows; nodes are assigned to chunks balancing summed
    src-degree, so each (window, chunk) edge group fits a fixed number of
    128-edge blocks (SPMD-uniform geometry across all cores).
  * MLP weights / BN parameters are replicated (folded into per-feature
    affine scale+bias on the host; O(D) work).

Per-core device pipeline (feature-major activations, [feat, node] tiles):
  1. dma_gather of x[src] rows (4 calls per 7-window granule, one per chunk),
  2. SWDGE accumulate-DMA adds edge_attr into the gathered tile,
  3. ScalarE relu -> messages,
  4. one-hot selection matrices S (VectorE iota-compare) turn the
     segment-sum into PE matmuls accumulated in PSUM:
         aggr[f, n] += sum_m msg[m, f] * S[m, n]
     plus an identity-matmul that adds (1+eps)*x (and transposes x to
     feature-major for free),
  5. MLP1 matmul + fused BN1+ReLU (ScalarE activation, per-partition affine),
     MLP2 matmul + identity-matmul residual + fused BN2+ReLU,
  6. PE transpose back to node-major, DMA out.
"""

import numpy as np

import concourse.bass as bass
import concourse.bacc as bacc
import concourse.mybir as mybir
import concourse.tile as tile
from concourse.bass_utils import run_bass_kernel_spmd

# ---------------------------------------------------------------- constants
N_NODES = 100000
D = 128
P = 128                      # partitions
NCORES = 8
NW = 98                      # 128-node windows per core
BPC = NW * P                 # padded nodes per core (12544)
NPAD = NCORES * BPC          # padded node table rows (100352)
WG = 7                       # windows per granule (pipeline unit)
NCHUNKS = 1                  # single gather table (int32 indirect DMA)
BN_EPS = 1e-5

_NC_CACHE: dict = {}
LAST_RESULTS = None          # BassKernelResults of the most recent run


# ------------------------------------------------------------- host planning
def _lpt_pack(deg_desc, nbins, cap_nodes):
    """Assign nodes (given in degree-descending order) to nbins bins of
    <=cap_nodes nodes each, greedily balancing summed degree per bin.
    Returns (bin_of_node, slot_of_node, loads)."""
    n = len(deg_desc)
    loads = np.zeros(nbins)
    cnt = np.zeros(nbins, np.int64)
    b_of = np.empty(n, np.int64)
    s_of = np.empty(n, np.int64)
    inf = np.inf
    for i in range(n):
        masked = np.where(cnt < cap_nodes, loads, inf)
        b = int(np.argmin(masked))
        b_of[i] = b
        s_of[i] = cnt[b]
        loads[b] += deg_desc[i]
        cnt[b] += 1
    return b_of, s_of, loads


def _plan(src, dst, n_nodes, ncores, nw, nchunks, ch):
    """Returns (pos_of_node, chunk_of_node, posc_of_node, kbc)."""
    bpc = nw * P
    per_core = n_nodes // ncores
    assert per_core * ncores == n_nodes and per_core <= bpc

    # --- destination side: core + window assignment by in-degree
    deg = np.bincount(dst, minlength=n_nodes)
    rank_order = np.argsort(-deg, kind="stable")
    ranks = np.arange(n_nodes)
    grp, off = divmod(ranks, ncores)
    core_of_rank = np.where(grp % 2 == 0, off, ncores - 1 - off)  # serpentine
    pos_of_node = np.empty(n_nodes, np.int64)
    deg_sorted = deg[rank_order]
    for c in range(ncores):
        m = core_of_rank == c
        w_of, s_of, _ = _lpt_pack(deg_sorted[m], nw, P)
        pos_of_node[rank_order[m]] = c * bpc + w_of * P + s_of

    # --- source side: gather-table chunk assignment by out-degree
    sdeg = np.bincount(src, minlength=n_nodes)
    sorder = np.argsort(-sdeg, kind="stable")
    c_of, s_of, _ = _lpt_pack(sdeg[sorder], nchunks, ch)
    chunk_of = np.empty(n_nodes, np.int64)
    chunk_of[sorder] = c_of
    posc_of = np.empty(n_nodes, np.int64)
    posc_of[sorder] = s_of

    # blocks per (window, chunk)
    cnt = np.bincount((pos_of_node[dst] // P) * nchunks + chunk_of[src],
                      minlength=ncores * nw * nchunks)
    kbc = max(2, int(np.ceil(cnt.max() / P)))
    return pos_of_node, chunk_of, posc_of, kbc


# ------------------------------------------------------------- device build
def _build(nw, wg, kbc, nchunks, ch, npad):
    """Build the per-core Bass program. All cores run this same program on
    different data."""
    ng = nw // wg                # granules
    cpc = wg * kbc               # gx columns per (granule, chunk)
    cpg = nchunks * cpc          # gx columns per granule
    nbc = ng * cpg               # block-columns per core
    nidx = nbc                   # one int32 index column per block
    f32 = mybir.dt.float32

    nc = bacc.Bacc(None)
    xg = nc.dram_tensor("xg", [npad, D], f32, kind="ExternalInput")
    ea = nc.dram_tensor("ea", [P, nbc * D], f32, kind="ExternalInput")
    xo = nc.dram_tensor("xo", [P, nw * D], f32, kind="ExternalInput")
    idx = nc.dram_tensor("idx", [P, nidx], mybir.dt.int32,
                         kind="ExternalInput")
    dstrel = nc.dram_tensor("dstrel", [P, nbc], f32, kind="ExternalInput")
    iotac = nc.dram_tensor("iotac", [P, P], f32, kind="ExternalInput")
    ideps = nc.dram_tensor("ideps", [P, P], f32, kind="ExternalInput")
    iden = nc.dram_tensor("iden", [P, P], f32, kind="ExternalInput")
    w1t = nc.dram_tensor("w1t", [D, D], f32, kind="ExternalInput")
    w2t = nc.dram_tensor("w2t", [D, D], f32, kind="ExternalInput")
    ab1 = nc.dram_tensor("ab1", [D, 2], f32, kind="ExternalInput")
    ab2 = nc.dram_tensor("ab2", [D, 2], f32, kind="ExternalInput")
    out = nc.dram_tensor("out", [P, nw * D], f32, kind="ExternalOutput")

    relu = mybir.ActivationFunctionType.Relu
    addop = mybir.AluOpType.add
    iseq = mybir.AluOpType.is_equal
    ts = bass.ts

    with tile.TileContext(nc) as tc:
        with (
            tc.tile_pool(name="const", bufs=1) as cp,
            tc.tile_pool(name="gx", bufs=6) as gxp,
            tc.tile_pool(name="ea", bufs=2) as eap,
            tc.tile_pool(name="sel", bufs=2) as sp,
            tc.tile_pool(name="xot", bufs=2) as xop,
            tc.tile_pool(name="z", bufs=4) as zp,
            tc.tile_pool(name="u", bufs=4) as up,
            tc.tile_pool(name="ofm", bufs=4) as ofp,
            tc.tile_pool(name="osb", bufs=2) as osp,
            tc.tile_pool(name="pz", bufs=2, space="PSUM") as pzp,
            tc.tile_pool(name="ph", bufs=2, space="PSUM") as php,
            tc.tile_pool(name="p2", bufs=2, space="PSUM") as p2p,
            tc.tile_pool(name="pt", bufs=2, space="PSUM") as ptp,
        ):
            # resident tensors
            idx_t = cp.tile([P, nidx], mybir.dt.int32)
            nc.sync.dma_start(out=idx_t[:, :], in_=idx[:, :])
            dst_t = cp.tile([P, nbc], f32)
            nc.sync.dma_start(out=dst_t[:, :], in_=dstrel[:, :])
            iota_t = cp.tile([P, P], f32)
            nc.sync.dma_start(out=iota_t[:, :], in_=iotac[:, :])
            ideps_t = cp.tile([P, P], f32)
            nc.sync.dma_start(out=ideps_t[:, :], in_=ideps[:, :])
            iden_t = cp.tile([P, P], f32)
            nc.sync.dma_start(out=iden_t[:, :], in_=iden[:, :])
            w1t_t = cp.tile([D, D], f32)
            nc.sync.dma_start(out=w1t_t[:, :], in_=w1t[:, :])
            w2t_t = cp.tile([D, D], f32)
            nc.sync.dma_start(out=w2t_t[:, :], in_=w2t[:, :])
            ab1_t = cp.tile([D, 2], f32)
            nc.sync.dma_start(out=ab1_t[:, :], in_=ab1[:, :])
            ab2_t = cp.tile([D, 2], f32)
            nc.sync.dma_start(out=ab2_t[:, :], in_=ab2[:, :])

            for g in range(ng):
                # ---- messages: gather x[src] per chunk, += edge_attr, relu
                # one indirect DMA per 128-edge block: HW honours
                # exactly one dynamic row index per partition per
                # instruction.  Per-block tiles from a 6-deep pool bound
                # the in-flight SWDGE descriptors (6*128 = 768) below the
                # ~1024-descriptor dynamic-DMA ring capacity; edge_attr
                # streams via HWDGE and is added on VectorE instead of a
                # SWDGE accumulate-DMA for the same reason.
                ea_t = eap.tile([P, cpg * D], f32)
                nc.sync.dma_start(
                    out=ea_t[:, :],
                    in_=ea[:, g * cpg * D:(g + 1) * cpg * D],
                )
                gxts = []
                for col in range(cpg):
                    gcol = g * cpg + col
                    gxb = gxp.tile([P, D], f32, tag="gxb")
                    nc.gpsimd.indirect_dma_start(
                        out=gxb[:, :],
                        out_offset=None,
                        in_=xg[:, :],
                        in_offset=bass.IndirectOffsetOnAxis(
                            ap=idx_t[:, gcol:gcol + 1], axis=0),
                    )
                    nc.vector.tensor_add(
                        out=gxb[:, :], in0=gxb[:, :],
                        in1=ea_t[:, ts(col, D)])
                    nc.scalar.activation(
                        out=gxb[:, :], in_=gxb[:, :], func=relu)
                    gxts.append(gxb)

                # ---- own nodes (node-major x rows for this granule)
                xot = xop.tile([P, wg * D], f32)
                nc.sync.dma_start(
                    out=xot[:, :], in_=xo[:, g * wg * D:(g + 1) * wg * D]
                )

                # ---- one-hot selection matrices for the whole granule
                sel = sp.tile([P, cpg * D], f32)
                for c in range(nchunks):
                    col0 = g * cpg + c * cpc
                    in0 = (
                        dst_t[:, col0:col0 + cpc]
                        .rearrange("p (j o) -> p j o", o=1)
                        .to_broadcast([P, cpc, P])
                    )
                    in1 = (
                        iota_t[:, :]
                        .rearrange("p (o n) -> p o n", o=1)
                        .to_broadcast([P, cpc, P])
                    )
                    # The S3S3D3_TT ISA struct only holds ONE sync wait;
                    # Bacc.compile()'s generate_event_semaphores splits any
                    # excess waits into standalone EventSemaphore ops.
                    nc.vector.tensor_tensor(
                        out=sel[:, c * cpc * D:(c + 1) * cpc * D]
                        .rearrange("p (j n) -> p j n", j=cpc),
                        in0=in0,
                        in1=in1,
                        op=iseq,
                    )

                osb = osp.tile([P, wg * D], f32)
                for wi in range(wg):
                    # ---- aggregation: z[f,n] = (1+eps)x + sum(msg) in PSUM
                    pz = pzp.tile([P, P], f32, space="PSUM")
                    mm = 0
                    for c in range(nchunks):
                        for b in range(kbc):
                            col = c * cpc + wi * kbc + b
                            nc.tensor.matmul(
                                out=pz[:, :],
                                lhsT=gxts[col][:, :],
                                rhs=sel[:, ts(col, D)],
                                start=(mm == 0),
                                stop=False,
                            )
                            mm += 1
                    nc.tensor.matmul(
                        out=pz[:, :],
                        lhsT=xot[:, ts(wi, D)],
                        rhs=ideps_t[:, :],
                        start=False,
                        stop=True,
                    )
                    z = zp.tile([P, P], f32)
                    nc.vector.tensor_copy(out=z[:, :], in_=pz[:, :])

                    # ---- MLP layer 1 + BN1 + relu
                    ph = php.tile([P, P], f32, space="PSUM")
                    nc.tensor.matmul(
                        out=ph[:, :], lhsT=w1t_t[:, :], rhs=z[:, :],
                        start=True, stop=True,
                    )
                    u = up.tile([P, P], f32)
                    nc.scalar.activation(
                        out=u[:, :], in_=ph[:, :], func=relu,
                        scale=ab1_t[:, 0:1], bias=ab1_t[:, 1:2],
                    )

                    # ---- MLP layer 2 + residual + BN2 + relu
                    p2 = p2p.tile([P, P], f32, space="PSUM")
                    nc.tensor.matmul(
                        out=p2[:, :], lhsT=w2t_t[:, :], rhs=u[:, :],
                        start=True, stop=False,
                    )
                    nc.tensor.matmul(
                        out=p2[:, :], lhsT=xot[:, ts(wi, D)],
                        rhs=iden_t[:, :], start=False, stop=True,
                    )
                    ofm = ofp.tile([P, P], f32)
                    nc.scalar.activation(
                        out=ofm[:, :], in_=p2[:, :], func=relu,
                        scale=ab2_t[:, 0:1], bias=ab2_t[:, 1:2],
                    )

                    # ---- back to node-major
                    pt = ptp.tile([P, P], f32, space="PSUM")
                    nc.tensor.transpose(
                        out=pt[:, :], in_=ofm[:, :], identity=iden_t[:, :]
                    )
                    nc.vector.tensor_copy(out=osb[:, ts(wi, D)], in_=pt[:, :])

                nc.sync.dma_start(
                    out=out[:, g * wg * D:(g + 1) * wg * D], in_=osb[:, :]
                )

    nc.compile()
    return nc


def _get_nc(key):
    if key not in _NC_CACHE:
        _NC_CACHE[key] = _build(*key)
    return _NC_CACHE[key]


# --------------------------------------------------------------- host driver
def _prepare(x, edge_index, edge_attr, eps, W1, b1, g1, bt1, rm1, rv1,
             W2, b2, g2, bt2, rm2, rv2, n_nodes, ncores, nw, wg, nchunks):
    """Shard + reformat all inputs.
    Returns (in_maps, kbc, ch, pos_of_node)."""
    bpc = nw * P
    npad = ncores * bpc
    ch = npad // nchunks
    src = np.asarray(edge_index[0], dtype=np.int64)
    dst = np.asarray(edge_index[1], dtype=np.int64)
    e = len(src)

    pos_of_node, chunk_of, posc_of, kbc = _plan(
        src, dst, n_nodes, ncores, nw, nchunks, ch)

    ng = nw // wg
    cpc = wg * kbc
    cpg = nchunks * cpc
    nbc = ng * cpg

    # --- edge -> slot: group by (core-window, chunk), pad to kbc blocks
    src_p = pos_of_node[src]
    dst_p = pos_of_node[dst]
    wgid = dst_p // P                       # global window id
    cid = chunk_of[src]
    gid = wgid * nchunks + cid              # (window, chunk) group
    order = np.argsort(gid, kind="stable")
    counts = np.bincount(gid, minlength=ncores * nw * nchunks)
    assert counts.max() <= kbc * P, (counts.max(), kbc * P)
    starts = np.zeros(ncores * nw * nchunks, np.int64)
    np.cumsum(counts[:-1], out=starts[1:])
    offs = np.arange(e, dtype=np.int64) - starts[gid[order]]

    # slot -> (global column, partition): group g=(core,w,c) occupies kbc
    # columns; within core, col = gg*cpg + c*cpc + wi*kbc + b
    og = gid[order]
    core_o = og // (nw * nchunks)
    w_o = (og // nchunks) % nw
    c_o = og % nchunks
    gg_o, wi_o = np.divmod(w_o, wg)
    col = (core_o * nbc + gg_o * cpg + c_o * cpc + wi_o * kbc + offs // P)
    prt = offs % P

    tot_cols = ncores * nbc
    srcidx_full = np.zeros((tot_cols, P), np.int32)
    srcidx_full[col, prt] = (chunk_of[src[order]] * ch
                             + posc_of[src[order]]).astype(np.int32)
    dstrel_full = np.full((tot_cols, P), -1.0, np.float32)
    dstrel_full[col, prt] = (dst_p[order] % P).astype(np.float32)
    ea_full = np.zeros((tot_cols, P, D), np.float32)
    ea_full[col, prt] = np.asarray(edge_attr, dtype=np.float32)[order]

    # --- device layouts
    # idx: per call (g,c): flat n = col_in_call*128+p ; sbuf[p, s] =
    # flat[s*16 + p%16], replicated over partition groups of 16
    idx_dev = np.ascontiguousarray(
        srcidx_full.reshape(ncores, nbc, P).transpose(0, 2, 1))

    dstrel_c = np.ascontiguousarray(
        dstrel_full.reshape(ncores, nbc, P).transpose(0, 2, 1))
    ea_c = np.ascontiguousarray(
        ea_full.reshape(ncores, nbc, P, D).transpose(0, 2, 1, 3)
        .reshape(ncores, P, nbc * D))

    # gather table (chunk-major, by src position)
    xg = np.zeros((npad, D), np.float32)
    xg[chunk_of * ch + posc_of] = np.asarray(x, dtype=np.float32)

    # own-node rows (window-major, by dst position)
    xperm = np.zeros((npad, D), np.float32)
    xperm[pos_of_node] = np.asarray(x, dtype=np.float32)
    xo_c = np.ascontiguousarray(
        xperm.reshape(ncores, nw, P, D).transpose(0, 2, 1, 3)
        .reshape(ncores, P, nw * D))

    # --- replicated constants
    epsf = float(np.asarray(eps))
    iotac = np.tile(np.arange(P, dtype=np.float32), (P, 1))
    ideps = ((1.0 + epsf) * np.eye(P)).astype(np.float32)
    iden = np.eye(P, dtype=np.float32)
    w1tm = np.ascontiguousarray(np.asarray(W1, np.float32).T)
    w2tm = np.ascontiguousarray(np.asarray(W2, np.float32).T)
    inv1 = 1.0 / np.sqrt(np.asarray(rv1, np.float32) + BN_EPS)
    a1 = np.asarray(g1, np.float32) * inv1
    beta1 = a1 * np.asarray(b1, np.float32) + np.asarray(bt1, np.float32) \
        - np.asarray(rm1, np.float32) * a1
    inv2 = 1.0 / np.sqrt(np.asarray(rv2, np.float32) + BN_EPS)
    a2 = np.asarray(g2, np.float32) * inv2
    beta2 = a2 * np.asarray(b2, np.float32) + np.asarray(bt2, np.float32) \
        - np.asarray(rm2, np.float32) * a2
    ab1 = np.ascontiguousarray(np.stack([a1, beta1], 1).astype(np.float32))
    ab2 = np.ascontiguousarray(np.stack([a2, beta2], 1).astype(np.float32))

    in_maps = []
    for c in range(ncores):
        in_maps.append({
            "xg": xg,
            "ea": ea_c[c],
            "xo": xo_c[c],
            "idx": idx_dev[c],
            "dstrel": dstrel_c[c],
            "iotac": iotac,
            "ideps": ideps,
            "iden": iden,
            "w1t": w1tm,
            "w2t": w2tm,
            "ab1": ab1,
            "ab2": ab2,
        })
    return in_maps, kbc, ch, pos_of_node


def kernel(**inputs) -> np.ndarray:
    global LAST_RESULTS
    x = np.asarray(inputs["x"], dtype=np.float32)
    n_nodes = x.shape[0]
    assert n_nodes == N_NODES and x.shape[1] == D

    in_maps, kbc, ch, pos_of_node = _prepare(
        x, inputs["edge_index"], inputs["edge_attr_emb"], inputs["eps"],
        inputs["W1"], inputs["b1"], inputs["g1"], inputs["bt1"],
        inputs["rm1"], inputs["rv1"],
        inputs["W2"], inputs["b2"], inputs["g2"], inputs["bt2"],
        inputs["rm2"], inputs["rv2"],
        n_nodes, NCORES, NW, WG, NCHUNKS,
    )
    nc = _get_nc((NW, WG, kbc, NCHUNKS, ch, NPAD))
    res = run_bass_kernel_spmd(nc, in_maps, core_ids=list(range(NCORES)))
    LAST_RESULTS = res

    # out[c] is [P, NW*D] partition-major; slot (p, w*D + f) = padded node
    # row c*BPC + w*P + p
    outp = np.stack([res.results[c]["out"] for c in range(NCORES)])
    out_rows = outp.reshape(NCORES, P, NW, D).transpose(0, 2, 1, 3) \
        .reshape(NPAD, D)
    return np.ascontiguousarray(out_rows[pos_of_node])



# revision 2
# speedup vs baseline: 1.4576x; 1.4576x over previous
"""Trainium2 Bass kernel for a GINE message-passing layer.

Reference computation (N=100000 nodes, E=600000 edges, D=128):
    msg  = relu(x[src] + edge_attr)            # [E, D]
    aggr = segment_sum(msg, dst, N)            # [N, D]
    z    = (1 + eps) * x + aggr
    h    = relu(bn1(z @ W1.T + b1)) @ W2.T + b2
    out  = relu(bn2(x + h))

Distribution strategy (8 NeuronCores, host-side shard/unshard):
  * Nodes partitioned across cores (graph parallel); node->core and
    node->window assignment degree-balanced (LPT) so every 128-node
    window receives ~E/(8*98) = 765 incoming edges.
  * Edges assigned to the core owning their destination; segment-sum is
    core-local.  Each window's edges are split across 4 gather "chunks"
    with static capacities {256,256,128,128} (= {2,2,1,1} blocks of
    128), so window capacity is 768 slots (99.6% fill).
  * Per (core, chunk) a private gather table holds the fp32 x-rows of
    the srcs referenced by that chunk's edges (<= 25088 rows, so the
    int16 dma_gather index reaches them).  The halo exchange is a host
    side copy into these tables; the gather itself runs on device.
  * x[src] rows are fetched with SWDGE dma_gather, <=512 indices per
    call (the SWDGE descriptor ring holds 1024, so two calls pipeline
    per ring), round-robin over all 4 SWDGE queues.  The gather is
    descriptor-latency-bound (~10ns/row/ring), so fp32 512B rows cost
    the same as bf16 and avoid sub-512B SBUF read-modify-write.
  * Messages/selection/MLP run in bf16 (PSUM accumulation fp32); the
    output is streamed back bf16 and widened on the host.  MLP weights
    and BN parameters are replicated; BN is folded into per-feature
    scale+bias applied by the ScalarE activation.

Per-core device pipeline (feature-major activations, [feat, node] tiles),
granule = 7 windows (42 gather blocks, 5376 edge slots):
  1. dma_gather x[src] (12 calls, 4 SWDGE queues), HWDGE edge_attr,
  2. VectorE add + ScalarE relu -> messages (bf16),
  3. VectorE iota-compare builds the one-hot selection S (bf16),
  4. per window: 6 PE matmuls msg@S + 1 identity-matmul (adds (1+eps)x,
     transposing x to feature-major for free) accumulate z in a
     4-window [128,512] PSUM group,
  5. per group: MLP1 + fused BN1+ReLU, MLP2 + per-window residual
     identity-matmuls + BN2+ReLU (ScalarE, batched per group),
  6. PE transpose back to node-major into a granule-wide bf16 PSUM
     tile, single ScalarE copy, DMA out (bf16).
"""

import numpy as np
import ml_dtypes

import concourse.bass as bass
import concourse.bacc as bacc
import concourse.mybir as mybir
import concourse.tile as tile
from concourse.bass_utils import run_bass_kernel_spmd

# ---------------------------------------------------------------- constants
N_NODES = 100000
D = 128
P = 128                      # partitions
NCORES = 8
NW = 98                      # 128-node windows per core
BPC = NW * P                 # padded nodes per core (12544)
NPAD = NCORES * BPC          # padded node table rows (100352)
WG = 7                       # windows per granule (pipeline unit)
NG = NW // WG                # granules (14)
KBC = (2, 2, 1, 1)           # gather blocks per (window, chunk)
NCHUNKS = len(KBC)
CAPW = 128 * sum(KBC)        # edge slots per window (768)
CH = NW * 128 * max(KBC)     # rows per (core, chunk) gather table (25088)
CPG = WG * sum(KBC)          # gather blocks per granule (42)
NBC = NG * CPG               # blocks per core (588)
CBASE = tuple(int(x) for x in  # block-column base of each chunk in granule
              np.concatenate([[0], np.cumsum([WG * k for k in KBC])[:-1]]))
GCALL = 512                  # max idxs per dma_gather call (ring holds two)
CALLS = []                   # (chunk, first block, nblocks) per granule call
for _c, _k in enumerate(KBC):
    _tot = WG * _k
    _b = 0
    while _b < _tot:
        _nb = min(GCALL // 128, _tot - _b)
        CALLS.append((_c, _b, _nb))
        _b += _nb
ICOLS = sum(_nb * 128 // 16 for _, _, _nb in CALLS)  # idx cols per granule
NSWQ = 4                     # SWDGE queues (4 rings double the gather rate)
GROUPS = ((0, 4), (4, 3))    # (first window, n windows) PSUM batches
BN_EPS = 1e-5
BF16 = ml_dtypes.bfloat16

_NC_CACHE: dict = {}
LAST_RESULTS = None          # BassKernelResults of the most recent run


# ------------------------------------------------------------- host planning
def _lpt_pack(deg_desc, nbins, cap_nodes):
    """Assign nodes (in degree-descending order) to nbins bins of
    <=cap_nodes nodes, greedily balancing summed degree per bin."""
    n = len(deg_desc)
    loads = np.zeros(nbins)
    cnt = np.zeros(nbins, np.int64)
    b_of = np.empty(n, np.int64)
    s_of = np.empty(n, np.int64)
    inf = np.inf
    for i in range(n):
        masked = np.where(cnt < cap_nodes, loads, inf)
        b = int(np.argmin(masked))
        b_of[i] = b
        s_of[i] = cnt[b]
        loads[b] += deg_desc[i]
        cnt[b] += 1
    return b_of, s_of, loads


def _plan_windows(dst, n_nodes):
    """LPT node->position assignment. Returns pos_of_node."""
    deg = np.bincount(dst, minlength=n_nodes)
    rank_order = np.argsort(-deg, kind="stable")
    ranks = np.arange(n_nodes)
    grp, off = divmod(ranks, NCORES)
    core_of_rank = np.where(grp % 2 == 0, off, NCORES - 1 - off)  # serpentine
    pos_of_node = np.empty(n_nodes, np.int64)
    deg_sorted = deg[rank_order]
    for c in range(NCORES):
        m = core_of_rank == c
        w_of, s_of, _ = _lpt_pack(deg_sorted[m], NW, P)
        pos_of_node[rank_order[m]] = c * BPC + w_of * P + s_of
    return pos_of_node


# ------------------------------------------------------------- device build
def _build():
    f32 = mybir.dt.float32
    bf16 = mybir.dt.bfloat16
    i16 = mybir.dt.int16

    nc = bacc.Bacc(None, num_swdge_queues=NSWQ)
    xg = nc.dram_tensor("xg", [NCHUNKS * CH, D], f32, kind="ExternalInput")
    ea = nc.dram_tensor("ea", [P, NBC * D], bf16, kind="ExternalInput")
    xo = nc.dram_tensor("xo", [P, NW * D], bf16, kind="ExternalInput")
    idx = nc.dram_tensor("idx", [P, NG * ICOLS], i16, kind="ExternalInput")
    dstrel = nc.dram_tensor("dstrel", [P, NBC], bf16, kind="ExternalInput")
    iotac = nc.dram_tensor("iotac", [P, P], bf16, kind="ExternalInput")
    ideps = nc.dram_tensor("ideps", [P, P], bf16, kind="ExternalInput")
    iden = nc.dram_tensor("iden", [P, P], bf16, kind="ExternalInput")
    w1t = nc.dram_tensor("w1t", [D, D], bf16, kind="ExternalInput")
    w2t = nc.dram_tensor("w2t", [D, D], bf16, kind="ExternalInput")
    ab1 = nc.dram_tensor("ab1", [D, 2], f32, kind="ExternalInput")
    ab2 = nc.dram_tensor("ab2", [D, 2], f32, kind="ExternalInput")
    out = nc.dram_tensor("out", [P, NW * D], bf16, kind="ExternalOutput")

    relu = mybir.ActivationFunctionType.Relu
    ident = mybir.ActivationFunctionType.Copy
    addop = mybir.AluOpType.add
    iseq = mybir.AluOpType.is_equal
    ts = bass.ts

    with tile.TileContext(nc) as tc:
        with (
            tc.tile_pool(name="const", bufs=1) as cp,
            tc.tile_pool(name="gx", bufs=3) as gxp,
            tc.tile_pool(name="msg", bufs=2) as msgp,
            tc.tile_pool(name="ea", bufs=2) as eap,
            tc.tile_pool(name="sel", bufs=2) as sp,
            tc.tile_pool(name="z", bufs=2) as zp,
            tc.tile_pool(name="u", bufs=2) as up,
            tc.tile_pool(name="ofm", bufs=2) as ofp,
            tc.tile_pool(name="osb", bufs=2) as osp,
            tc.tile_pool(name="pz", bufs=2, space="PSUM") as pzp,
            tc.tile_pool(name="ph", bufs=2, space="PSUM") as php,
            tc.tile_pool(name="p2", bufs=2, space="PSUM") as p2p,
            tc.tile_pool(name="pt", bufs=2, space="PSUM") as ptp,
        ):
            # resident tensors
            idx_t = cp.tile([P, NG * ICOLS], i16)
            nc.sync.dma_start(out=idx_t[:, :], in_=idx[:, :])
            dst_t = cp.tile([P, NBC], bf16)
            nc.sync.dma_start(out=dst_t[:, :], in_=dstrel[:, :])
            iota_t = cp.tile([P, P], bf16)
            nc.sync.dma_start(out=iota_t[:, :], in_=iotac[:, :])
            ideps_t = cp.tile([P, P], bf16)
            nc.sync.dma_start(out=ideps_t[:, :], in_=ideps[:, :])
            iden_t = cp.tile([P, P], bf16)
            nc.sync.dma_start(out=iden_t[:, :], in_=iden[:, :])
            w1t_t = cp.tile([D, D], bf16)
            nc.sync.dma_start(out=w1t_t[:, :], in_=w1t[:, :])
            w2t_t = cp.tile([D, D], bf16)
            nc.sync.dma_start(out=w2t_t[:, :], in_=w2t[:, :])
            ab1_t = cp.tile([D, 2], f32)
            nc.sync.dma_start(out=ab1_t[:, :], in_=ab1[:, :])
            ab2_t = cp.tile([D, 2], f32)
            nc.sync.dma_start(out=ab2_t[:, :], in_=ab2[:, :])
            xo_t = cp.tile([P, NW * D], bf16)
            nc.sync.dma_start(out=xo_t[:, :], in_=xo[:, :])

            for g in range(NG):
                # ---- gather x[src] (fp32 rows; descriptor-latency-bound,
                # so fp32 costs the same as bf16)
                gx = gxp.tile([P, CPG * D], f32)
                io = g * ICOLS
                for cidx, (c, b0, nb) in enumerate(CALLS):
                    col0 = CBASE[c] + b0
                    n = nb * 128
                    nc.gpsimd.dma_gather(
                        gx[:, col0 * D:(col0 + nb) * D]
                        .rearrange("p (j e) -> p j e", e=D),
                        xg[c * CH:(c + 1) * CH, :],
                        idx_t[:, io:io + n // 16],
                        n, n, D,
                        queue_num=(g * len(CALLS) + cidx) % NSWQ,
                    )
                    io += n // 16

                # ---- edge attrs, messages = relu(gx + ea) -> bf16
                ea_t = eap.tile([P, CPG * D], bf16)
                nc.sync.dma_start(
                    out=ea_t[:, :],
                    in_=ea[:, g * CPG * D:(g + 1) * CPG * D],
                )
                msg = msgp.tile([P, CPG * D], bf16)
                nc.vector.tensor_tensor(out=msg[:, :], in0=gx[:, :],
                                        in1=ea_t[:, :], op=addop)
                nc.scalar.activation(out=msg[:, :], in_=msg[:, :], func=relu)

                # ---- one-hot selection matrices for the whole granule
                sel = sp.tile([P, CPG * D], bf16)
                in0 = (
                    dst_t[:, g * CPG:(g + 1) * CPG]
                    .rearrange("p (j o) -> p j o", o=1)
                    .to_broadcast([P, CPG, P])
                )
                in1 = (
                    iota_t[:, :]
                    .rearrange("p (o n) -> p o n", o=1)
                    .to_broadcast([P, CPG, P])
                )
                nc.vector.tensor_tensor(
                    out=sel[:, :].rearrange("p (j n) -> p j n", j=CPG),
                    in0=in0, in1=in1, op=iseq,
                )

                zt = zp.tile([P, WG * D], bf16)
                osb = osp.tile([P, WG * D], bf16)
                pt = ptp.tile([P, WG * D], bf16, space="PSUM")
                for w0, nw in GROUPS:
                    gc = nw * P          # columns in this group
                    # ---- aggregation: z[f,n] = (1+eps)x + sum(msg), PSUM
                    pz = pzp.tile([P, gc], f32, space="PSUM")
                    for wr in range(nw):
                        wi = w0 + wr
                        mm = 0
                        for c in range(NCHUNKS):
                            for b in range(KBC[c]):
                                col = CBASE[c] + wi * KBC[c] + b
                                nc.tensor.matmul(
                                    out=pz[:, ts(wr, P)],
                                    lhsT=msg[:, ts(col, D)],
                                    rhs=sel[:, ts(col, D)],
                                    start=(mm == 0),
                                    stop=False,
                                )
                                mm += 1
                        nc.tensor.matmul(
                            out=pz[:, ts(wr, P)],
                            lhsT=xo_t[:, ts(g * WG + wi, D)],
                            rhs=ideps_t[:, :],
                            start=False,
                            stop=True,
                        )
                    nc.scalar.activation(
                        out=zt[:, w0 * P:w0 * P + gc], in_=pz[:, :],
                        func=ident,
                    )

                    # ---- MLP layer 1 + BN1 + relu (whole group)
                    ph = php.tile([P, gc], f32, space="PSUM")
                    nc.tensor.matmul(
                        out=ph[:, :], lhsT=w1t_t[:, :],
                        rhs=zt[:, w0 * P:w0 * P + gc],
                        start=True, stop=True,
                    )
                    u = up.tile([P, gc], bf16)
                    nc.scalar.activation(
                        out=u[:, :], in_=ph[:, :], func=relu,
                        scale=ab1_t[:, 0:1], bias=ab1_t[:, 1:2],
                    )

                    # ---- MLP layer 2 + residual + BN2 + relu
                    p2 = p2p.tile([P, gc], f32, space="PSUM")
                    nc.tensor.matmul(
                        out=p2[:, :], lhsT=w2t_t[:, :], rhs=u[:, :],
                        start=True, stop=False,
                    )
                    for wr in range(nw):
                        nc.tensor.matmul(
                            out=p2[:, ts(wr, P)],
                            lhsT=xo_t[:, ts(g * WG + w0 + wr, D)],
                            rhs=iden_t[:, :], start=False, stop=True,
                        )
                    ofm = ofp.tile([P, gc], bf16)
                    nc.scalar.activation(
                        out=ofm[:, :], in_=p2[:, :], func=relu,
                        scale=ab2_t[:, 0:1], bias=ab2_t[:, 1:2],
                    )

                    # ---- back to node-major
                    for wr in range(nw):
                        nc.tensor.transpose(
                            out=pt[:, ts(w0 + wr, P)], in_=ofm[:, ts(wr, P)],
                            identity=iden_t[:, :],
                        )
                nc.scalar.activation(out=osb[:, :], in_=pt[:, :], func=ident)

                nc.sync.dma_start(
                    out=out[:, g * WG * D:(g + 1) * WG * D], in_=osb[:, :]
                )

    nc.compile()
    return nc


def _get_nc():
    if "nc" not in _NC_CACHE:
        _NC_CACHE["nc"] = _build()
    return _NC_CACHE["nc"]


# --------------------------------------------------------------- host driver
def _prepare(x, edge_index, edge_attr, eps, W1, b1, g1, bt1, rm1, rv1,
             W2, b2, g2, bt2, rm2, rv2):
    """Shard + reformat all inputs. Returns (in_maps, pos_of_node)."""
    src = np.asarray(edge_index[0], dtype=np.int64)
    dst = np.asarray(edge_index[1], dtype=np.int64)
    e = len(src)
    x32 = np.asarray(x, dtype=np.float32)
    xbf = x32.astype(BF16)
    ea32 = np.asarray(edge_attr, dtype=np.float32).astype(BF16)

    pos_of_node = _plan_windows(dst, N_NODES)

    # --- order edges by destination window, rank within window
    dst_p = pos_of_node[dst]
    wgid = dst_p // P                        # global window id (core*NW + w)
    order = np.argsort(wgid, kind="stable")
    counts = np.bincount(wgid, minlength=NCORES * NW)
    assert counts.max() <= CAPW, (counts.max(), CAPW)
    starts = np.zeros(NCORES * NW, np.int64)
    np.cumsum(counts[:-1], out=starts[1:])
    rank = np.arange(e, dtype=np.int64) - starts[wgid[order]]   # within window

    # --- chunk by rank (capacities 128*KBC), then slot within chunk
    caps = np.array([128 * k for k in KBC])
    cends = np.cumsum(caps)
    cstarts = cends - caps
    cid = np.searchsorted(cends, rank, side="right")
    rc = rank - cstarts[cid]                 # rank within (window, chunk)

    core_o = wgid[order] // NW
    w_o = wgid[order] % NW
    g_o, wi_o = np.divmod(w_o, WG)
    kbc_o = np.array(KBC)[cid]
    cbase_o = np.array(CBASE)[cid]
    col = (core_o * NBC + g_o * CPG + cbase_o + wi_o * kbc_o + rc // P)
    prt = rc % P

    # --- per-(core, chunk) gather tables and local src rows
    src_o = src[order]
    tot_cols = NCORES * NBC
    srcidx_full = np.zeros((tot_cols, P), np.int16)
    xg_c = np.zeros((NCORES, NCHUNKS * CH, D), np.float32)
    for c in range(NCORES):
        for ch in range(NCHUNKS):
            m = (core_o == c) & (cid == ch)
            uniq, inv = np.unique(src_o[m], return_inverse=True)
            assert len(uniq) <= CH, (c, ch, len(uniq))
            xg_c[c, ch * CH:ch * CH + len(uniq)] = x32[uniq]
            srcidx_full[col[m], prt[m]] = inv.astype(np.int16)

    dstrel_full = np.full((tot_cols, P), -1.0, BF16)
    dstrel_full[col, prt] = (dst_p[order] % P).astype(BF16)
    ea_full = np.zeros((tot_cols, P, D), BF16)
    ea_full[col, prt] = ea32[order]

    # --- device layouts
    srcidx_c = srcidx_full.reshape(NCORES, NBC, P)
    idx_dev = np.zeros((NCORES, P, NG * ICOLS), np.int16)
    for g in range(NG):
        io = g * ICOLS
        for c, b0, nb in CALLS:
            col0 = g * CPG + CBASE[c] + b0
            n = nb * 128
            flat = srcidx_c[:, col0:col0 + nb, :].reshape(NCORES, n)
            blk = flat.reshape(NCORES, n // 16, 16).transpose(0, 2, 1)
            idx_dev[:, :, io:io + n // 16] = np.tile(blk, (1, 8, 1))
            io += n // 16

    dstrel_c = np.ascontiguousarray(
        dstrel_full.reshape(NCORES, NBC, P).transpose(0, 2, 1))
    ea_c = np.ascontiguousarray(
        ea_full.reshape(NCORES, NBC, P, D).transpose(0, 2, 1, 3)
        .reshape(NCORES, P, NBC * D))

    # own-node rows (window-major, by dst position)
    xperm = np.zeros((NPAD, D), BF16)
    xperm[pos_of_node] = xbf
    xo_c = np.ascontiguousarray(
        xperm.reshape(NCORES, NW, P, D).transpose(0, 2, 1, 3)
        .reshape(NCORES, P, NW * D))

    # --- replicated constants
    epsf = float(np.asarray(eps))
    iotac = np.tile(np.arange(P, dtype=np.float32), (P, 1)).astype(BF16)
    ideps = ((1.0 + epsf) * np.eye(P)).astype(BF16)
    iden = np.eye(P, dtype=np.float32).astype(BF16)
    w1tm = np.ascontiguousarray(np.asarray(W1, np.float32).T).astype(BF16)
    w2tm = np.ascontiguousarray(np.asarray(W2, np.float32).T).astype(BF16)
    inv1 = 1.0 / np.sqrt(np.asarray(rv1, np.float32) + BN_EPS)
    a1 = np.asarray(g1, np.float32) * inv1
    beta1 = a1 * np.asarray(b1, np.float32) + np.asarray(bt1, np.float32) \
        - np.asarray(rm1, np.float32) * a1
    inv2 = 1.0 / np.sqrt(np.asarray(rv2, np.float32) + BN_EPS)
    a2 = np.asarray(g2, np.float32) * inv2
    beta2 = a2 * np.asarray(b2, np.float32) + np.asarray(bt2, np.float32) \
        - np.asarray(rm2, np.float32) * a2
    ab1 = np.ascontiguousarray(np.stack([a1, beta1], 1).astype(np.float32))
    ab2 = np.ascontiguousarray(np.stack([a2, beta2], 1).astype(np.float32))

    in_maps = []
    for c in range(NCORES):
        in_maps.append({
            "xg": xg_c[c],
            "ea": ea_c[c],
            "xo": xo_c[c],
            "idx": idx_dev[c],
            "dstrel": dstrel_c[c],
            "iotac": iotac,
            "ideps": ideps,
            "iden": iden,
            "w1t": w1tm,
            "w2t": w2tm,
            "ab1": ab1,
            "ab2": ab2,
        })
    return in_maps, pos_of_node


def kernel(**inputs) -> np.ndarray:
    global LAST_RESULTS
    x = np.asarray(inputs["x"], dtype=np.float32)
    assert x.shape == (N_NODES, D)

    in_maps, pos_of_node = _prepare(
        x, inputs["edge_index"], inputs["edge_attr_emb"], inputs["eps"],
        inputs["W1"], inputs["b1"], inputs["g1"], inputs["bt1"],
        inputs["rm1"], inputs["rv1"],
        inputs["W2"], inputs["b2"], inputs["g2"], inputs["bt2"],
        inputs["rm2"], inputs["rv2"],
    )
    nc = _get_nc()
    res = run_bass_kernel_spmd(nc, in_maps, core_ids=list(range(NCORES)))
    LAST_RESULTS = res

    # out[c] is [P, NW*D] partition-major; slot (p, w*D + f) = padded node
    # row c*BPC + w*P + p
    outp = np.stack([np.asarray(res.results[c]["out"]) for c in range(NCORES)])
    out_rows = outp.reshape(NCORES, P, NW, D).transpose(0, 2, 1, 3) \
        .reshape(NPAD, D)
    return np.ascontiguousarray(out_rows[pos_of_node]).astype(np.float32)


# revision 3
# speedup vs baseline: 2.0571x; 1.4113x over previous
"""Trainium2 Bass kernel for a GINE message-passing layer.

Reference computation (N=100000 nodes, E=600000 edges, D=128):
    msg  = relu(x[src] + edge_attr)            # [E, D]
    aggr = segment_sum(msg, dst, N)            # [N, D]
    z    = (1 + eps) * x + aggr
    h    = relu(bn1(z @ W1.T + b1)) @ W2.T + b2
    out  = relu(bn2(x + h))

Distribution strategy (8 NeuronCores, host-side shard/unshard):
  * Nodes partitioned across cores (graph parallel); node->core and
    node->window assignment degree-balanced (LPT) so every 128-node
    window receives ~E/(8*98) = 765 incoming edges.
  * Edges assigned to the core owning their destination; segment-sum is
    core-local.  Each window's edges are split across 4 gather "chunks"
    with static capacities {256,256,128,128} (= {2,2,1,1} blocks of
    128), so window capacity is 768 slots (99.6% fill).
  * Per (core, chunk) a private gather table holds the fp32 x-rows of
    the srcs referenced by that chunk's edges (<= 25088 rows, so the
    int16 dma_gather index reaches them).  The halo exchange is a host
    side copy into these tables; the gather itself runs on device.
  * x[src] rows are fetched with SWDGE dma_gather, <=896 indices per
    call (a 32 KiB descriptor carveout gives 2048-desc rings, so two
    calls pipeline per ring), round-robin over all 4 SWDGE queues.  The gather is
    descriptor-latency-bound (~10ns/row/ring), so fp32 512B rows cost
    the same as bf16 and avoid sub-512B SBUF read-modify-write.
  * Messages/selection/MLP run in bf16 (PSUM accumulation fp32); the
    output is streamed back bf16 and widened on the host.  MLP weights
    and BN parameters are replicated; BN is folded into per-feature
    scale+bias applied by the ScalarE activation.

Per-core device pipeline (feature-major activations, [feat, node] tiles),
granule = 7 windows (42 gather blocks, 5376 edge slots):
  1. dma_gather x[src] (12 calls, 4 SWDGE queues), HWDGE edge_attr,
  2. VectorE add + ScalarE relu -> messages (bf16),
  3. VectorE iota-compare builds the one-hot selection S (bf16),
  4. per window: 6 PE matmuls msg@S + 1 identity-matmul (adds (1+eps)x,
     transposing x to feature-major for free) accumulate z in a
     4-window [128,512] PSUM group,
  5. per group: MLP1 + fused BN1+ReLU, MLP2 + per-window residual
     identity-matmuls + BN2+ReLU (ScalarE, batched per group),
  6. PE transpose back to node-major into a granule-wide bf16 PSUM
     tile, single ScalarE copy, DMA out (bf16).
"""

import numpy as np
import ml_dtypes

import concourse.bass as bass
import concourse.bacc as bacc
import concourse.mybir as mybir
import concourse.tile as tile
from concourse.bass_utils import run_bass_kernel_spmd

# ---------------------------------------------------------------- constants
N_NODES = 100000
D = 128
P = 128                      # partitions
NCORES = 8
NW = 98                      # 128-node windows per core
BPC = NW * P                 # padded nodes per core (12544)
NPAD = NCORES * BPC          # padded node table rows (100352)
WG = 7                       # windows per granule (pipeline unit)
NG = NW // WG                # granules (14)
KBC = (2, 2, 1, 1)           # gather blocks per (window, chunk)
NCHUNKS = len(KBC)
CAPW = 128 * sum(KBC)        # edge slots per window (768)
CH = NW * 128 * max(KBC)     # rows per (core, chunk) gather table (25088)
CPG = WG * sum(KBC)          # gather blocks per granule (42)
NBC = NG * CPG               # blocks per core (588)
CBASE = tuple(int(x) for x in  # block-column base of each chunk in granule
              np.concatenate([[0], np.cumsum([WG * k for k in KBC])[:-1]]))
GCALL = 896                  # max idxs per dma_gather call (2048-desc ring holds two)
CALLS = []                   # (chunk, first block, nblocks) per granule call
for _c, _k in enumerate(KBC):
    _tot = WG * _k
    _b = 0
    while _b < _tot:
        _nb = min(GCALL // 128, _tot - _b)
        CALLS.append((_c, _b, _nb))
        _b += _nb
ICOLS = sum(_nb * 128 // 16 for _, _, _nb in CALLS)  # idx cols per granule
NSWQ = 4                     # SWDGE queues (4 rings double the gather rate)
GROUPS = ((0, 4), (4, 3))    # (first window, n windows) PSUM batches
BN_EPS = 1e-5
BF16 = ml_dtypes.bfloat16

_NC_CACHE: dict = {}
LAST_RESULTS = None          # BassKernelResults of the most recent run


# ------------------------------------------------------------- host planning
def _lpt_pack(deg_desc, nbins, cap_nodes):
    """Assign nodes (in degree-descending order) to nbins bins of
    <=cap_nodes nodes, greedily balancing summed degree per bin."""
    n = len(deg_desc)
    loads = np.zeros(nbins)
    cnt = np.zeros(nbins, np.int64)
    b_of = np.empty(n, np.int64)
    s_of = np.empty(n, np.int64)
    inf = np.inf
    for i in range(n):
        masked = np.where(cnt < cap_nodes, loads, inf)
        b = int(np.argmin(masked))
        b_of[i] = b
        s_of[i] = cnt[b]
        loads[b] += deg_desc[i]
        cnt[b] += 1
    return b_of, s_of, loads


def _plan_windows(dst, n_nodes):
    """LPT node->position assignment. Returns pos_of_node."""
    deg = np.bincount(dst, minlength=n_nodes)
    rank_order = np.argsort(-deg, kind="stable")
    ranks = np.arange(n_nodes)
    grp, off = divmod(ranks, NCORES)
    core_of_rank = np.where(grp % 2 == 0, off, NCORES - 1 - off)  # serpentine
    pos_of_node = np.empty(n_nodes, np.int64)
    deg_sorted = deg[rank_order]
    for c in range(NCORES):
        m = core_of_rank == c
        w_of, s_of, _ = _lpt_pack(deg_sorted[m], NW, P)
        pos_of_node[rank_order[m]] = c * BPC + w_of * P + s_of
    return pos_of_node


# ------------------------------------------------------------- device build
def _build():
    f32 = mybir.dt.float32
    bf16 = mybir.dt.bfloat16
    i16 = mybir.dt.int16

    nc = bacc.Bacc(None, num_swdge_queues=NSWQ,
                   dynamic_dma_scratch_size=32768)
    xg = nc.dram_tensor("xg", [NCHUNKS * CH, D], f32, kind="ExternalInput")
    ea = nc.dram_tensor("ea", [P, NBC * D], bf16, kind="ExternalInput")
    xo = nc.dram_tensor("xo", [P, NW * D], bf16, kind="ExternalInput")
    idx = nc.dram_tensor("idx", [P, NG * ICOLS], i16, kind="ExternalInput")
    dstrel = nc.dram_tensor("dstrel", [P, NBC], bf16, kind="ExternalInput")
    iotac = nc.dram_tensor("iotac", [P, P], bf16, kind="ExternalInput")
    ideps = nc.dram_tensor("ideps", [P, P], bf16, kind="ExternalInput")
    iden = nc.dram_tensor("iden", [P, P], bf16, kind="ExternalInput")
    w1t = nc.dram_tensor("w1t", [D, D], bf16, kind="ExternalInput")
    w2t = nc.dram_tensor("w2t", [D, D], bf16, kind="ExternalInput")
    ab1 = nc.dram_tensor("ab1", [D, 2], f32, kind="ExternalInput")
    ab2 = nc.dram_tensor("ab2", [D, 2], f32, kind="ExternalInput")
    out = nc.dram_tensor("out", [P, NW * D], bf16, kind="ExternalOutput")

    relu = mybir.ActivationFunctionType.Relu
    ident = mybir.ActivationFunctionType.Copy
    addop = mybir.AluOpType.add
    iseq = mybir.AluOpType.is_equal
    ts = bass.ts

    with tile.TileContext(nc) as tc:
        with (
            tc.tile_pool(name="const", bufs=1) as cp,
            tc.tile_pool(name="gx", bufs=3) as gxp,
            tc.tile_pool(name="msg", bufs=2) as msgp,
            tc.tile_pool(name="ea", bufs=2) as eap,
            tc.tile_pool(name="sel", bufs=2) as sp,
            tc.tile_pool(name="z", bufs=2) as zp,
            tc.tile_pool(name="u", bufs=2) as up,
            tc.tile_pool(name="ofm", bufs=2) as ofp,
            tc.tile_pool(name="osb", bufs=2) as osp,
            tc.tile_pool(name="pz", bufs=2, space="PSUM") as pzp,
            tc.tile_pool(name="ph", bufs=2, space="PSUM") as php,
            tc.tile_pool(name="p2", bufs=2, space="PSUM") as p2p,
            tc.tile_pool(name="pt", bufs=2, space="PSUM") as ptp,
        ):
            # resident tensors
            idx_t = cp.tile([P, NG * ICOLS], i16)
            nc.sync.dma_start(out=idx_t[:, :], in_=idx[:, :])
            dst_t = cp.tile([P, NBC], bf16)
            nc.sync.dma_start(out=dst_t[:, :], in_=dstrel[:, :])
            iota_t = cp.tile([P, P], bf16)
            nc.sync.dma_start(out=iota_t[:, :], in_=iotac[:, :])
            ideps_t = cp.tile([P, P], bf16)
            nc.sync.dma_start(out=ideps_t[:, :], in_=ideps[:, :])
            iden_t = cp.tile([P, P], bf16)
            nc.sync.dma_start(out=iden_t[:, :], in_=iden[:, :])
            w1t_t = cp.tile([D, D], bf16)
            nc.sync.dma_start(out=w1t_t[:, :], in_=w1t[:, :])
            w2t_t = cp.tile([D, D], bf16)
            nc.sync.dma_start(out=w2t_t[:, :], in_=w2t[:, :])
            ab1_t = cp.tile([D, 2], f32)
            nc.sync.dma_start(out=ab1_t[:, :], in_=ab1[:, :])
            ab2_t = cp.tile([D, 2], f32)
            nc.sync.dma_start(out=ab2_t[:, :], in_=ab2[:, :])
            xo_t = cp.tile([P, NW * D], bf16)
            nc.sync.dma_start(out=xo_t[:, :], in_=xo[:, :])

            for g in range(NG):
                # ---- gather x[src] (fp32 rows; descriptor-latency-bound,
                # so fp32 costs the same as bf16)
                gx = gxp.tile([P, CPG * D], f32)
                io = g * ICOLS
                for cidx, (c, b0, nb) in enumerate(CALLS):
                    col0 = CBASE[c] + b0
                    n = nb * 128
                    nc.gpsimd.dma_gather(
                        gx[:, col0 * D:(col0 + nb) * D]
                        .rearrange("p (j e) -> p j e", e=D),
                        xg[c * CH:(c + 1) * CH, :],
                        idx_t[:, io:io + n // 16],
                        n, n, D,
                        queue_num=(g * len(CALLS) + cidx) % NSWQ,
                    )
                    io += n // 16

                # ---- edge attrs, messages = relu(gx + ea) -> bf16
                ea_t = eap.tile([P, CPG * D], bf16)
                nc.sync.dma_start(
                    out=ea_t[:, :],
                    in_=ea[:, g * CPG * D:(g + 1) * CPG * D],
                )
                msg = msgp.tile([P, CPG * D], bf16)
                nc.vector.tensor_tensor(out=msg[:, :], in0=gx[:, :],
                                        in1=ea_t[:, :], op=addop)
                nc.scalar.activation(out=msg[:, :], in_=msg[:, :], func=relu)

                # ---- one-hot selection matrices for the whole granule
                sel = sp.tile([P, CPG * D], bf16)
                in0 = (
                    dst_t[:, g * CPG:(g + 1) * CPG]
                    .rearrange("p (j o) -> p j o", o=1)
                    .to_broadcast([P, CPG, P])
                )
                in1 = (
                    iota_t[:, :]
                    .rearrange("p (o n) -> p o n", o=1)
                    .to_broadcast([P, CPG, P])
                )
                nc.vector.tensor_tensor(
                    out=sel[:, :].rearrange("p (j n) -> p j n", j=CPG),
                    in0=in0, in1=in1, op=iseq,
                )

                zt = zp.tile([P, WG * D], bf16)
                osb = osp.tile([P, WG * D], bf16)
                pt = ptp.tile([P, WG * D], bf16, space="PSUM")
                for w0, nw in GROUPS:
                    gc = nw * P          # columns in this group
                    # ---- aggregation: z[f,n] = (1+eps)x + sum(msg), PSUM
                    pz = pzp.tile([P, gc], f32, space="PSUM")
                    for wr in range(nw):
                        wi = w0 + wr
                        mm = 0
                        for c in range(NCHUNKS):
                            for b in range(KBC[c]):
                                col = CBASE[c] + wi * KBC[c] + b
                                nc.tensor.matmul(
                                    out=pz[:, ts(wr, P)],
                                    lhsT=msg[:, ts(col, D)],
                                    rhs=sel[:, ts(col, D)],
                                    start=(mm == 0),
                                    stop=False,
                                )
                                mm += 1
                        nc.tensor.matmul(
                            out=pz[:, ts(wr, P)],
                            lhsT=xo_t[:, ts(g * WG + wi, D)],
                            rhs=ideps_t[:, :],
                            start=False,
                            stop=True,
                        )
                    nc.scalar.activation(
                        out=zt[:, w0 * P:w0 * P + gc], in_=pz[:, :],
                        func=ident,
                    )

                    # ---- MLP layer 1 + BN1 + relu (whole group)
                    ph = php.tile([P, gc], f32, space="PSUM")
                    nc.tensor.matmul(
                        out=ph[:, :], lhsT=w1t_t[:, :],
                        rhs=zt[:, w0 * P:w0 * P + gc],
                        start=True, stop=True,
                    )
                    u = up.tile([P, gc], bf16)
                    nc.scalar.activation(
                        out=u[:, :], in_=ph[:, :], func=relu,
                        scale=ab1_t[:, 0:1], bias=ab1_t[:, 1:2],
                    )

                    # ---- MLP layer 2 + residual + BN2 + relu
                    p2 = p2p.tile([P, gc], f32, space="PSUM")
                    nc.tensor.matmul(
                        out=p2[:, :], lhsT=w2t_t[:, :], rhs=u[:, :],
                        start=True, stop=False,
                    )
                    for wr in range(nw):
                        nc.tensor.matmul(
                            out=p2[:, ts(wr, P)],
                            lhsT=xo_t[:, ts(g * WG + w0 + wr, D)],
                            rhs=iden_t[:, :], start=False, stop=True,
                        )
                    ofm = ofp.tile([P, gc], bf16)
                    nc.scalar.activation(
                        out=ofm[:, :], in_=p2[:, :], func=relu,
                        scale=ab2_t[:, 0:1], bias=ab2_t[:, 1:2],
                    )

                    # ---- back to node-major
                    for wr in range(nw):
                        nc.tensor.transpose(
                            out=pt[:, ts(w0 + wr, P)], in_=ofm[:, ts(wr, P)],
                            identity=iden_t[:, :],
                        )
                nc.scalar.activation(out=osb[:, :], in_=pt[:, :], func=ident)

                nc.sync.dma_start(
                    out=out[:, g * WG * D:(g + 1) * WG * D], in_=osb[:, :]
                )

    nc.compile()
    return nc


def _get_nc():
    if "nc" not in _NC_CACHE:
        _NC_CACHE["nc"] = _build()
    return _NC_CACHE["nc"]


# --------------------------------------------------------------- host driver
def _prepare(x, edge_index, edge_attr, eps, W1, b1, g1, bt1, rm1, rv1,
             W2, b2, g2, bt2, rm2, rv2):
    """Shard + reformat all inputs. Returns (in_maps, pos_of_node)."""
    src = np.asarray(edge_index[0], dtype=np.int64)
    dst = np.asarray(edge_index[1], dtype=np.int64)
    e = len(src)
    x32 = np.asarray(x, dtype=np.float32)
    xbf = x32.astype(BF16)
    ea32 = np.asarray(edge_attr, dtype=np.float32).astype(BF16)

    pos_of_node = _plan_windows(dst, N_NODES)

    # --- order edges by destination window, rank within window
    dst_p = pos_of_node[dst]
    wgid = dst_p // P                        # global window id (core*NW + w)
    order = np.argsort(wgid, kind="stable")
    counts = np.bincount(wgid, minlength=NCORES * NW)
    assert counts.max() <= CAPW, (counts.max(), CAPW)
    starts = np.zeros(NCORES * NW, np.int64)
    np.cumsum(counts[:-1], out=starts[1:])
    rank = np.arange(e, dtype=np.int64) - starts[wgid[order]]   # within window

    # --- chunk by rank (capacities 128*KBC), then slot within chunk
    caps = np.array([128 * k for k in KBC])
    cends = np.cumsum(caps)
    cstarts = cends - caps
    cid = np.searchsorted(cends, rank, side="right")
    rc = rank - cstarts[cid]                 # rank within (window, chunk)

    core_o = wgid[order] // NW
    w_o = wgid[order] % NW
    g_o, wi_o = np.divmod(w_o, WG)
    kbc_o = np.array(KBC)[cid]
    cbase_o = np.array(CBASE)[cid]
    col = (core_o * NBC + g_o * CPG + cbase_o + wi_o * kbc_o + rc // P)
    prt = rc % P

    # --- per-(core, chunk) gather tables and local src rows
    src_o = src[order]
    tot_cols = NCORES * NBC
    srcidx_full = np.zeros((tot_cols, P), np.int16)
    xg_c = np.zeros((NCORES, NCHUNKS * CH, D), np.float32)
    for c in range(NCORES):
        for ch in range(NCHUNKS):
            m = (core_o == c) & (cid == ch)
            uniq, inv = np.unique(src_o[m], return_inverse=True)
            assert len(uniq) <= CH, (c, ch, len(uniq))
            xg_c[c, ch * CH:ch * CH + len(uniq)] = x32[uniq]
            srcidx_full[col[m], prt[m]] = inv.astype(np.int16)

    dstrel_full = np.full((tot_cols, P), -1.0, BF16)
    dstrel_full[col, prt] = (dst_p[order] % P).astype(BF16)
    ea_full = np.zeros((tot_cols, P, D), BF16)
    ea_full[col, prt] = ea32[order]

    # --- device layouts
    srcidx_c = srcidx_full.reshape(NCORES, NBC, P)
    idx_dev = np.zeros((NCORES, P, NG * ICOLS), np.int16)
    for g in range(NG):
        io = g * ICOLS
        for c, b0, nb in CALLS:
            col0 = g * CPG + CBASE[c] + b0
            n = nb * 128
            flat = srcidx_c[:, col0:col0 + nb, :].reshape(NCORES, n)
            blk = flat.reshape(NCORES, n // 16, 16).transpose(0, 2, 1)
            idx_dev[:, :, io:io + n // 16] = np.tile(blk, (1, 8, 1))
            io += n // 16

    dstrel_c = np.ascontiguousarray(
        dstrel_full.reshape(NCORES, NBC, P).transpose(0, 2, 1))
    ea_c = np.ascontiguousarray(
        ea_full.reshape(NCORES, NBC, P, D).transpose(0, 2, 1, 3)
        .reshape(NCORES, P, NBC * D))

    # own-node rows (window-major, by dst position)
    xperm = np.zeros((NPAD, D), BF16)
    xperm[pos_of_node] = xbf
    xo_c = np.ascontiguousarray(
        xperm.reshape(NCORES, NW, P, D).transpose(0, 2, 1, 3)
        .reshape(NCORES, P, NW * D))

    # --- replicated constants
    epsf = float(np.asarray(eps))
    iotac = np.tile(np.arange(P, dtype=np.float32), (P, 1)).astype(BF16)
    ideps = ((1.0 + epsf) * np.eye(P)).astype(BF16)
    iden = np.eye(P, dtype=np.float32).astype(BF16)
    w1tm = np.ascontiguousarray(np.asarray(W1, np.float32).T).astype(BF16)
    w2tm = np.ascontiguousarray(np.asarray(W2, np.float32).T).astype(BF16)
    inv1 = 1.0 / np.sqrt(np.asarray(rv1, np.float32) + BN_EPS)
    a1 = np.asarray(g1, np.float32) * inv1
    beta1 = a1 * np.asarray(b1, np.float32) + np.asarray(bt1, np.float32) \
        - np.asarray(rm1, np.float32) * a1
    inv2 = 1.0 / np.sqrt(np.asarray(rv2, np.float32) + BN_EPS)
    a2 = np.asarray(g2, np.float32) * inv2
    beta2 = a2 * np.asarray(b2, np.float32) + np.asarray(bt2, np.float32) \
        - np.asarray(rm2, np.float32) * a2
    ab1 = np.ascontiguousarray(np.stack([a1, beta1], 1).astype(np.float32))
    ab2 = np.ascontiguousarray(np.stack([a2, beta2], 1).astype(np.float32))

    in_maps = []
    for c in range(NCORES):
        in_maps.append({
            "xg": xg_c[c],
            "ea": ea_c[c],
            "xo": xo_c[c],
            "idx": idx_dev[c],
            "dstrel": dstrel_c[c],
            "iotac": iotac,
            "ideps": ideps,
            "iden": iden,
            "w1t": w1tm,
            "w2t": w2tm,
            "ab1": ab1,
            "ab2": ab2,
        })
    return in_maps, pos_of_node


def kernel(**inputs) -> np.ndarray:
    global LAST_RESULTS
    x = np.asarray(inputs["x"], dtype=np.float32)
    assert x.shape == (N_NODES, D)

    in_maps, pos_of_node = _prepare(
        x, inputs["edge_index"], inputs["edge_attr_emb"], inputs["eps"],
        inputs["W1"], inputs["b1"], inputs["g1"], inputs["bt1"],
        inputs["rm1"], inputs["rv1"],
        inputs["W2"], inputs["b2"], inputs["g2"], inputs["bt2"],
        inputs["rm2"], inputs["rv2"],
    )
    nc = _get_nc()
    res = run_bass_kernel_spmd(nc, in_maps, core_ids=list(range(NCORES)))
    LAST_RESULTS = res

    # out[c] is [P, NW*D] partition-major; slot (p, w*D + f) = padded node
    # row c*BPC + w*P + p
    outp = np.stack([np.asarray(res.results[c]["out"]) for c in range(NCORES)])
    out_rows = outp.reshape(NCORES, P, NW, D).transpose(0, 2, 1, 3) \
        .reshape(NPAD, D)
    return np.ascontiguousarray(out_rows[pos_of_node]).astype(np.float32)
